# revision 1
# baseline (speedup 1.0000x reference)
"""GCNConv (DGL GraphConv norm='both') on 8 Trainium2 NeuronCores.

out = D_dst^-1/2 * A * (D_src^-1/2 * X * W) + b
  X: [100000, 32] f32, edge_index: [2, 1600000] (src, dst), W: [32, 32], b: [32]

Sharding: nodes are range-partitioned across the 8 cores (12500 each); each
core owns the aggregation for dst nodes in its range (graph/data parallel).
The host only buckets/sorts/remaps integer indices (graph sharding + layout);
all floating-point math runs on device.

Device algorithm (single SPMD program):
  Phase 1: core k computes m = (x_k @ W) * outdeg^-1/2 for its own node range
           into 256B-padded f16 rows, then AllGather -> full m on every core.
  Phase 2: edges are pre-sorted by dst. Each 128-dst window's edges are split
           by src quarter (4 groups) and padded per (window, group) to a
           common block count R_q (SPMD-uniform). Messages m[src] are fetched
           with dma_gather (elem = 256B row, per-quarter table slice so the
           int16 index fits). Per window, a one-hot matrix (edge -> local
           dst) is built in one vector-engine is_equal op and the 4*R_q
           accumulating matmuls produce the window's aggregation directly in
           PSUM. Scale by indeg^-1/2 (device-computed from CSR indptr
           diffs), add bias.
"""

import os
import sys

import numpy as np

for _p in ("/opt/trn_rl_repo", "/root/.axon_site/_ro/trn_rl_repo"):
    if os.path.isdir(_p) and _p not in sys.path:
        sys.path.insert(0, _p)

N_NODES = 100000
N_CORES = 8
NPC = N_NODES // N_CORES  # 12500 nodes per core
DIN = 32
DOUT = 32
P = 128  # partitions
MROW = 128  # f16 elements per padded m row (256 bytes)
NTILE = (NPC + P - 1) // P  # 98 dst windows per core
NPAD = NTILE * P
NG = 4  # src quarters

SPAN_W = 6  # windows per gather span


def _build_program(RQ):
    """Build the SPMD program. RQ = 128-edge blocks per (window, quarter)."""
    from concourse import bacc, bass, mybir, tile

    f32 = mybir.dt.float32
    f16 = mybir.dt.float16
    i16 = mybir.dt.int16
    i32 = mybir.dt.int32
    Alu = mybir.AluOpType
    Act = mybir.ActivationFunctionType

    QN = N_NODES // NG         # nodes per quarter (int16-addressable)
    span_w = max(1, 30 // RQ)  # windows per gather span (SBUF-bounded)
    WSL = NG * RQ              # slots per window
    SLOTS = NTILE * WSL        # 128-edge blocks per core
    GSL = NTILE * RQ           # slots per quarter-group
    nspan = (NTILE + span_w - 1) // span_w

    nc = bacc.Bacc(
        "TRN2",
        target_bir_lowering=False,
        debug=False,
        enable_asserts=False,
        num_devices=N_CORES,
    )

    # ---- I/O ----
    x_pad = nc.dram_tensor("x_pad", [NPAD, DIN], f32, kind="ExternalInput")
    w_in = nc.dram_tensor("w_in", [DIN, DOUT], f32, kind="ExternalInput")
    b_rep = nc.dram_tensor("b_rep", [P, DOUT], f32, kind="ExternalInput")
    # per-quarter gather indices, span-major ((w,g) run of RQ*128 each)
    qidx = [
        nc.dram_tensor(f"qidx{g}", [P, GSL * 8], i16, kind="ExternalInput")
        for g in range(NG)
    ]
    dstloc = nc.dram_tensor("dstloc", [P, SLOTS], f16, kind="ExternalInput")
    iota_in = nc.dram_tensor("iota_in", [P, P], f16, kind="ExternalInput")
    dA = nc.dram_tensor("dA", [P, NTILE], i32, kind="ExternalInput")
    dB = nc.dram_tensor("dB", [P, NTILE], i32, kind="ExternalInput")
    oA = nc.dram_tensor("oA", [P, NTILE], i32, kind="ExternalInput")
    oB = nc.dram_tensor("oB", [P, NTILE], i32, kind="ExternalInput")
    out_d = nc.dram_tensor("out_d", [NPAD, DOUT], f32, kind="ExternalOutput")

    # ---- internal DRAM ----
    m_own = nc.dram_tensor("m_own", [NPC, MROW], f16, kind="Internal")
    m_full = nc.dram_tensor(
        "m_full", [N_NODES, MROW], f16, kind="Internal", addr_space="Shared"
    )

    with tile.TileContext(nc) as tc:
        with (
            tc.tile_pool(name="const", bufs=1) as cpool,
            tc.tile_pool(name="work", bufs=3) as wpool,
            tc.tile_pool(name="gath", bufs=2) as gpool,
            tc.tile_pool(name="psum", bufs=4, space="PSUM") as ppool,
            tc.tile_pool(name="psum2", bufs=2, space="PSUM") as ppool2,
        ):
            # ---- load constants ----
            w_t = cpool.tile([DIN, DOUT], f32)
            nc.sync.dma_start(out=w_t[:], in_=w_in[:])
            b_t = cpool.tile([P, DOUT], f32)
            nc.sync.dma_start(out=b_t[:], in_=b_rep[:])
            iota_t = cpool.tile([P, P], f16)
            nc.sync.dma_start(out=iota_t[:], in_=iota_in[:])
            qidx_t = []
            for g in range(NG):
                t = cpool.tile([P, GSL * 8], i16, tag=f"qidx{g}")
                nc.sync.dma_start(out=t[:], in_=qidx[g][:])
                qidx_t.append(t)
            dst_t = cpool.tile([P, SLOTS], f16)
            nc.sync.dma_start(out=dst_t[:], in_=dstloc[:])
            idx_t = {}
            for nm, h in (("dA", dA), ("dB", dB), ("oA", oA), ("oB", oB)):
                t = cpool.tile([P, NTILE], i32, tag=nm)
                nc.sync.dma_start(out=t[:], in_=h[:])
                idx_t[nm] = t

            # ---- out-degree norm (node-major l = n*128 + p) ----
            ns_all = cpool.tile([P, NTILE], f32)
            odeg = wpool.tile([P, NTILE], f32, tag="odeg")
            nc.vector.tensor_tensor(
                out=odeg[:], in0=idx_t["oB"][:], in1=idx_t["oA"][:],
                op=Alu.subtract,
            )
            nc.vector.tensor_scalar_max(out=odeg[:], in0=odeg[:], scalar1=1.0)
            osq = wpool.tile([P, NTILE], f32, tag="osq")
            nc.scalar.activation(out=osq[:], in_=odeg[:], func=Act.Sqrt)
            nc.vector.reciprocal(out=ns_all[:], in_=osq[:])

            # ---- phase 1: m = (x @ W) * ns -> f16 padded rows ----
            G4 = 4
            for n0 in range(0, NTILE, G4):
                ng = min(G4, NTILE - n0)
                stg = wpool.tile([DIN, 4 * G4, DIN], f32, tag="stg")
                nc.sync.dma_start(
                    out=stg[:, :4 * ng, :],
                    in_=x_pad[n0 * P:(n0 + ng) * P, :].rearrange(
                        "(i p) q -> p i q", p=DIN
                    ),
                )
                xt = wpool.tile([DIN, G4 * P], f32, tag="xt")
                nc.vector.transpose(
                    out=xt[:, :ng * P],
                    in_=stg[:, :4 * ng, :].rearrange("p i q -> p (i q)"),
                )
                m_t = wpool.tile([P, G4, MROW], f16, tag="m_t")
                nc.vector.memset(m_t[:], 0.0)
                for j in range(ng):
                    n = n0 + j
                    hp = ppool2.tile([P, DOUT], f32)
                    nc.tensor.matmul(
                        out=hp[:], lhsT=xt[:, j * P:(j + 1) * P], rhs=w_t[:],
                        start=True, stop=True,
                    )
                    nc.vector.tensor_tensor(
                        out=m_t[:, j:j + 1, 0:DOUT], in0=hp[:].unsqueeze(1),
                        in1=ns_all[:, n:n + 1].unsqueeze(2)
                        .to_broadcast([P, 1, DOUT]),
                        op=Alu.mult,
                    )
                lo = n0 * P
                hi = min((n0 + ng) * P, NPC)
                full_tiles = (hi - lo) // P
                if full_tiles:
                    nc.sync.dma_start(
                        out=m_own[lo:lo + full_tiles * P, :].rearrange(
                            "(j p) c -> p j c", p=P
                        ),
                        in_=m_t[:, :full_tiles, :],
                    )
                rem = (hi - lo) - full_tiles * P
                if rem:
                    nc.sync.dma_start(
                        out=m_own[lo + full_tiles * P:hi, :],
                        in_=m_t[:rem, full_tiles, :],
                    )

            # ---- AllGather m ----
            nc.gpsimd.collective_compute(
                "AllGather",
                mybir.AluOpType.bypass,
                replica_groups=[list(range(N_CORES))],
                ins=[m_own[:]],
                outs=[m_full[:]],
            )

            # ---- phase 2: per-quarter gathers + windowed one-hot matmuls --
            out_stage = cpool.tile([P, NTILE, DOUT + 1], f32)

            q_tiles = [None] * nspan  # span -> [tile per group]

            def ensure_span(sp):
                if q_tiles[sp] is not None:
                    return
                w0 = sp * span_w
                nw = min(span_w, NTILE - w0)
                tiles = []
                for g in range(NG):
                    s0 = w0 * RQ          # slot offset within group-g stream
                    nsl = nw * RQ
                    n_idx = nsl * P
                    qt = gpool.tile([P, span_w * RQ, MROW], f16, tag=f"q{g}")
                    nc.gpsimd.dma_gather(
                        out_ap=qt[:, :nsl, :],
                        in_ap=m_full[g * QN:(g + 1) * QN, :],
                        idxs_ap=qidx_t[g][:, s0 * 8:(s0 + nsl) * 8],
                        num_idxs=n_idx,
                        num_idxs_reg=n_idx,
                        elem_size=MROW,
                        single_packet=False,
                    )
                    tiles.append(qt)
                q_tiles[sp] = tiles

            for w in range(NTILE):
                sp, wo = divmod(w, span_w)
                ensure_span(sp)
                if sp + 1 < nspan and wo == max(0, span_w - 2):
                    ensure_span(sp + 1)  # prefetch next span
                oh = wpool.tile([P, WSL, P + 1], f16, tag="onehot")
                nc.vector.tensor_tensor(
                    out=oh[:, :, 0:P],
                    in0=iota_t[:].unsqueeze(1).to_broadcast([P, WSL, P]),
                    in1=dst_t[:, w * WSL:(w + 1) * WSL]
                    .unsqueeze(2).to_broadcast([P, WSL, P]),
                    op=Alu.is_equal,
                )
                ps = ppool.tile([P, DOUT], f32)
                k = 0
                for g in range(NG):
                    qt = q_tiles[sp][g]
                    for r in range(RQ):
                        nc.tensor.matmul(
                            out=ps[:],
                            lhsT=oh[:, g * RQ + r, 0:P],
                            rhs=qt[:, wo * RQ + r, 0:DOUT],
                            start=(k == 0),
                            stop=(k == WSL - 1),
                        )
                        k += 1
                nc.scalar.activation(
                    out=out_stage[:, w:w + 1, 0:DOUT],
                    in_=ps[:].unsqueeze(1),
                    func=Act.Copy,
                )
                if wo == span_w - 1 or w == NTILE - 1:
                    q_tiles[sp] = None  # allow pool slot reuse

            # ---- final: scale by indeg^-1/2, add bias ----
            ideg = wpool.tile([P, NTILE], f32, tag="ideg")
            nc.vector.tensor_tensor(
                out=ideg[:], in0=idx_t["dB"][:], in1=idx_t["dA"][:],
                op=Alu.subtract,
            )
            nc.vector.tensor_scalar_max(out=ideg[:], in0=ideg[:], scalar1=1.0)
            isq = wpool.tile([P, NTILE], f32, tag="isq")
            nc.scalar.activation(out=isq[:], in_=ideg[:], func=Act.Sqrt)
            nd_all = wpool.tile([P, NTILE], f32, tag="nd")
            nc.vector.reciprocal(out=nd_all[:], in_=isq[:])

            outt = cpool.tile([P, NTILE, DOUT + 1], f32)
            nc.vector.tensor_tensor(
                out=outt[:, :, 0:DOUT], in0=out_stage[:, :, 0:DOUT],
                in1=nd_all[:].unsqueeze(2).to_broadcast([P, NTILE, DOUT]),
                op=Alu.mult,
            )
            nc.vector.tensor_tensor(
                out=outt[:, :, 0:DOUT], in0=outt[:, :, 0:DOUT],
                in1=b_t[:].unsqueeze(1).to_broadcast([P, NTILE, DOUT]),
                op=Alu.add,
            )
            # node l = w*128 + p
            nc.sync.dma_start(
                out=out_d[:].rearrange("(w p) c -> p w c", p=P),
                in_=outt[:, :, 0:DOUT],
            )

    nc.compile()
    return nc


def _preprocess(x, edge_index, W, b):
    """Host-side sharding: index-only bucketing/sorting/remapping."""
    src = np.asarray(edge_index[0], dtype=np.int64)
    dst = np.asarray(edge_index[1], dtype=np.int64)
    x = np.asarray(x, dtype=np.float32)
    W = np.asarray(W, dtype=np.float32)
    b = np.asarray(b, dtype=np.float32)

    QN = N_NODES // NG
    core_of = dst // NPC
    per_core = []
    rq_needed = 1
    for k in range(N_CORES):
        sel = core_of == k
        s_k = src[sel]
        d_k = dst[sel] - k * NPC
        # group edges by (window, src quarter), sorted
        win = d_k // P
        grp = s_k // QN
        order = np.lexsort((s_k, grp, win))
        s_k = s_k[order]
        d_k = d_k[order]
        win = win[order]
        grp = grp[order]
        # counts per (window, group)
        wg = win * NG + grp
        wg_counts = np.bincount(wg, minlength=NTILE * NG)
        rq_needed = max(rq_needed, int(np.ceil(wg_counts.max() / P)))
        counts = np.bincount(d_k, minlength=NPC)
        indptr = np.zeros(NPC + 1, dtype=np.int64)
        np.cumsum(counts, out=indptr[1:])
        per_core.append((s_k, d_k, wg_counts, indptr))

    RQ = int(rq_needed)
    WSL = NG * RQ
    SLOTS = NTILE * WSL
    GSL = NTILE * RQ

    iota_rep = np.broadcast_to(
        np.arange(P, dtype=np.float16)[None, :], (P, P)
    ).copy()
    b_rep = np.broadcast_to(b[None, :], (P, DOUT)).copy()

    in_maps = []
    for k in range(N_CORES):
        s_k, d_k, wg_counts, indptr = per_core[k]
        # slot layout: global slot s = w*WSL + g*RQ + r  (for dstloc/one-hot)
        # gather stream for group g: slot position w*RQ + r, edge j = pos*128+p
        e_src = np.zeros((NG, GSL * P), dtype=np.int64)  # per-group edge src
        e_dst = np.full(SLOTS * P, P, dtype=np.float16)  # local dst (pad=128)

        wg_starts = np.concatenate([[0], np.cumsum(wg_counts)])[:-1]
        n_e = len(s_k)
        pos_in_run = np.arange(n_e) - np.repeat(wg_starts, wg_counts)
        wv = np.repeat(np.arange(NTILE * NG) // NG, wg_counts)
        gv = np.repeat(np.arange(NTILE * NG) % NG, wg_counts)
        # per-group stream position
        jg = (wv * RQ) * P + pos_in_run
        e_src[gv, jg] = s_k - gv * QN
        # one-hot slot position
        js = (wv * WSL + gv * RQ) * P + pos_in_run
        e_dst[js] = (d_k - wv * P).astype(np.float16)

        qidx_arrs = {}
        for g in range(NG):
            flat = e_src[g].astype(np.int16)
            qi = flat.reshape(GSL * P // 16, 16).T
            qidx_arrs[f"qidx{g}"] = np.tile(qi, (8, 1))
        dstloc_arr = e_dst.reshape(SLOTS, P).T.copy()

        l_idx = np.arange(NPAD)
        valid = l_idx < NPC
        da = np.where(valid, indptr[np.minimum(l_idx, NPC - 1)], 0)
        db = np.where(valid, indptr[np.minimum(l_idx + 1, NPC)], 0)
        dA_ = da.astype(np.int32).reshape(NTILE, P).T.copy()
        dB_ = db.astype(np.int32).reshape(NTILE, P).T.copy()

        lo, hi = k * NPC, (k + 1) * NPC
        sel2 = (src >= lo) & (src < hi)
        ocounts = np.bincount(src[sel2] - lo, minlength=NPC)
        optr = np.zeros(NPC + 1, dtype=np.int64)
        np.cumsum(ocounts, out=optr[1:])
        oa = np.where(valid, optr[np.minimum(l_idx, NPC - 1)], 0)
        ob = np.where(valid, optr[np.minimum(l_idx + 1, NPC)], 0)
        oA_ = oa.astype(np.int32).reshape(NTILE, P).T.copy()
        oB_ = ob.astype(np.int32).reshape(NTILE, P).T.copy()

        x_k = np.zeros((NPAD, DIN), dtype=np.float32)
        x_k[:NPC] = x[lo:hi]

        in_maps.append({
            "x_pad": x_k, "w_in": W, "b_rep": b_rep,
            **qidx_arrs,
            "dstloc": dstloc_arr, "iota_in": iota_rep,
            "dA": dA_, "dB": dB_, "oA": oA_, "oB": oB_,
        })

    return in_maps, RQ


_prog_cache = {}
_last_results = None


def kernel(x, edge_index, W, b):
    from concourse import bass_utils

    in_maps, RQ = _preprocess(x, edge_index, W, b)
    if RQ not in _prog_cache:
        _prog_cache[RQ] = _build_program(RQ)
    nc = _prog_cache[RQ]

    res = bass_utils.run_bass_kernel_spmd(
        nc, in_maps, core_ids=list(range(N_CORES))
    )
    global _last_results
    _last_results = res
    outs = []
    for k in range(N_CORES):
        o = res.results[k]["out_d"]  # [NPAD, DOUT], node l = w*128 + p
        outs.append(o[:NPC])
    return np.concatenate(outs, axis=0).astype(np.float32)



# revision 2
# speedup vs baseline: 1.0428x; 1.0428x over previous
"""GCNConv (DGL GraphConv norm='both') on 8 Trainium2 NeuronCores.

out = D_dst^-1/2 * A * (D_src^-1/2 * X * W) + b
  X: [100000, 32] f32, edge_index: [2, 1600000] (src, dst), W: [32, 32], b: [32]

Rank-scatter design (src-sharded, zero gathers):
  Cores own src-node ranges (12500 each) and compute their messages
  m = (x * outdeg^-0.5) @ W entirely locally -- no exchange of messages.
  Aggregation runs through hardware DMA scatter-add: edges are split into
  lane streams j = dst & 3; a scatter call adds one message per node into
  accum_j[dst >> 2, 0:32] (f16 64B elements at 256B row stride, int16 row
  indices < 25088). The DMA scatter-add has no RMW atomicity, so a call
  must never contain duplicate rows: host assigns each edge a color
  (= call rank) via per-node bipartite matching constrained to colors
  [0, deg_j(node)) so call r covers exactly the sorted-node prefix with
  deg_j > r; unmatched edges go to a few offset-sliced overflow calls
  with rows kept distinct per call. Calls on different lane tensors
  pipeline; same-tensor calls serialize (which is required for
  correctness). Four small ReduceScatters (strided lane views) combine
  partials across cores; each core scales its node shard by indeg^-0.5
  and adds the bias.

Host does integer-only index work plus pure relayout of x (permutation +
transpose); all FP arithmetic runs on device.
"""

import os
import sys

import numpy as np

for _p in ("/opt/trn_rl_repo", "/root/.axon_site/_ro/trn_rl_repo"):
    if os.path.isdir(_p) and _p not in sys.path:
        sys.path.insert(0, _p)

N_NODES = 100000
N_CORES = 8
NPC = N_NODES // N_CORES      # 12500 own src nodes per core
DIN = 32
DOUT = 32
P = 128
NW = (NPC + P - 1) // P       # 98 blocks of own-node slots
NSL = NW * P                  # 12544 padded node slots
NLANE = 4                     # dst & 3 -> lane stream / accum tensor
R_REAL = N_NODES // NLANE     # 25000 accum rows
R_ROWS = 25088                # padded to /128; row 25000 = dump
DUMP = R_REAL
RS_N = R_REAL // N_CORES      # 3125 rows per core shard
MB_W = 14                     # matmul/scale batch (windows per psum bank)


def _build_program(call_plan):
    """call_plan: tuple of (j, a, nb) in issue order; in slots = blocks
    [a, a+nb) of m_j."""
    from concourse import bacc, bass, mybir, tile

    f32 = mybir.dt.float32
    f16 = mybir.dt.float16
    i16 = mybir.dt.int16
    i32 = mybir.dt.int32
    Alu = mybir.AluOpType
    Act = mybir.ActivationFunctionType

    tot_idx_cols = sum(nb * P // 16 for (_, _, nb) in call_plan)
    # lanes with a merged tail call (in slots = replica blocks at [NW, NW+T))
    tail_T = {}
    for (j, a, nb) in call_plan:
        if a == NW:
            tail_T[j] = nb

    nc = bacc.Bacc(
        "TRN2",
        target_bir_lowering=False,
        debug=False,
        enable_asserts=False,
        num_devices=N_CORES,
    )

    # ---- I/O ----
    xt_in = [
        nc.dram_tensor(f"xt{j}", [DIN, NSL], f32, kind="ExternalInput")
        for j in range(NLANE)
    ]
    odeg_in = [
        nc.dram_tensor(f"odeg{j}", [P, NW], i32, kind="ExternalInput")
        for j in range(NLANE)
    ]
    w_in = nc.dram_tensor("w_in", [DIN, DOUT], f32, kind="ExternalInput")
    b_rep = nc.dram_tensor("b_rep", [P, DOUT], f32, kind="ExternalInput")
    idx_in = nc.dram_tensor("idx_in", [P, tot_idx_cols], i16,
                            kind="ExternalInput")
    ideg_in = nc.dram_tensor("ideg_in", [P, NW], i32, kind="ExternalInput")
    out_d = nc.dram_tensor("out_d", [NPC, DOUT], f32, kind="ExternalOutput")

    # ---- internal DRAM ----
    accum = [
        nc.dram_tensor(f"accum{j}", [R_ROWS, P], f16, kind="Internal")
        for j in range(NLANE)
    ]
    dense = nc.dram_tensor("dense", [R_ROWS, P], f16, kind="Internal")
    rs_int = nc.dram_tensor("rs_int", [R_REAL // N_CORES, P], f16,
                            kind="Internal")

    with tile.TileContext(nc) as tc:
        with (
            tc.tile_pool(name="const", bufs=1) as cpool,
            tc.tile_pool(name="xt", bufs=2) as xpool,
            tc.tile_pool(name="work", bufs=3) as wpool,
            tc.tile_pool(name="psum", bufs=4, space="PSUM") as ppool,
        ):
            # ---- zero the used lanes of the accumulators ----
            ZB = 49  # blocks per zero chunk (196 total)
            zt = wpool.tile([P, ZB, DOUT], f16, tag="zt")
            nc.vector.memset(zt[:], 0.0)
            for j in range(NLANE):
                av = accum[j][:, 0:DOUT].rearrange("(b p) c -> p b c", p=P)
                for i in range(R_ROWS // P // ZB):
                    nc.sync.dma_start(out=av[:, i * ZB:(i + 1) * ZB, :],
                                      in_=zt[:])

            # ---- constants ----
            w_t = cpool.tile([DIN, DOUT], f32)
            nc.sync.dma_start(out=w_t[:], in_=w_in[:])
            b_t = cpool.tile([P, DOUT], f32)
            nc.sync.dma_start(out=b_t[:], in_=b_rep[:])
            idx_t = cpool.tile([P, tot_idx_cols], i16)
            nc.sync.dma_start(out=idx_t[:], in_=idx_in[:])

            # ---- phase 1: m_j = (x_j * ns_j) @ W for each lane order ----
            m_t = []
            for j in range(NLANE):
                dg = wpool.tile([P, NW], i32, tag="dg")
                nc.sync.dma_start(out=dg[:], in_=odeg_in[j][:])
                dgc = wpool.tile([P, NW], f32, tag="dgc")
                nc.vector.tensor_scalar_max(out=dgc[:], in0=dg[:], scalar1=1.0)
                sq = wpool.tile([P, NW], f32, tag="sq")
                nc.scalar.activation(out=sq[:], in_=dgc[:], func=Act.Sqrt)
                rsq = wpool.tile([P, NW], f32, tag="rsq")
                nc.vector.reciprocal(out=rsq[:], in_=sq[:])
                ind = wpool.tile([P, NW], f32, tag="ind")
                nc.vector.tensor_scalar_min(out=ind[:], in0=dg[:], scalar1=1.0)
                ns = wpool.tile([P, NW], f32, tag="ns")
                nc.vector.tensor_tensor(out=ns[:], in0=rsq[:], in1=ind[:],
                                        op=Alu.mult)

                TJ = tail_T.get(j, 0)
                mj = cpool.tile([P, NW + TJ, DOUT], f16, tag=f"m{j}")
                for w0 in range(0, NW, MB_W):
                    nw = min(MB_W, NW - w0)
                    xt_t = xpool.tile([DIN, MB_W * P], f32, tag="xt")
                    nc.sync.dma_start(
                        out=xt_t[:, 0:nw * P],
                        in_=xt_in[j][:, w0 * P:(w0 + nw) * P])
                    ps = ppool.tile([P, MB_W, DOUT], f32)
                    for i in range(nw):
                        nc.tensor.matmul(
                            out=ps[:, i, :],
                            lhsT=xt_t[:, i * P:(i + 1) * P],
                            rhs=w_t[:],
                            start=True, stop=True,
                        )
                    nc.vector.tensor_tensor(
                        out=mj[:, w0:w0 + nw, :],
                        in0=ps[:, 0:nw, :],
                        in1=ns[:, w0:w0 + nw].unsqueeze(2)
                        .to_broadcast([P, nw, DOUT]),
                        op=Alu.mult,
                    )
                # replica blocks for the merged tail call (copies of block 0)
                for t in range(TJ):
                    nc.vector.tensor_scalar_mul(
                        out=mj[:, NW + t:NW + t + 1, :],
                        in0=mj[:, 0:1, :],
                        scalar1=1.0,
                    )
                m_t.append(mj)

            # ---- phase 2: rank-scatter calls (dup-free rows per call) ----
            col = 0
            for (j, a, nb) in call_plan:
                ncols = nb * P // 16
                nc.gpsimd.dma_scatter_add(
                    accum[j][:, 0:DOUT],
                    m_t[j][:, a:a + nb, :],
                    idx_t[:, col:col + ncols],
                    nb * P,
                    nb * P,
                    DOUT,
                    elem_step=P,
                    single_packet=False,
                )
                col += ncols

            # ---- compact lanes into dense node-major accum, one RS ----
            for j in range(NLANE):
                nc.sync.dma_start(
                    out=dense[:, 32 * j:32 * j + DOUT],
                    in_=accum[j][:, 0:DOUT],
                )
            nc.gpsimd.collective_compute(
                "ReduceScatter",
                mybir.AluOpType.add,
                replica_groups=[list(range(N_CORES))],
                ins=[dense[0:R_REAL, :]],
                outs=[rs_int[:, :]],
            )

            # ---- finalize: scale by indeg^-1/2, add bias ----
            idg = wpool.tile([P, NW], i32, tag="idg")
            nc.sync.dma_start(out=idg[:], in_=ideg_in[:])
            idgc = wpool.tile([P, NW], f32, tag="idgc")
            nc.vector.tensor_scalar_max(out=idgc[:], in0=idg[:], scalar1=1.0)
            isq = wpool.tile([P, NW], f32, tag="isq")
            nc.scalar.activation(out=isq[:], in_=idgc[:], func=Act.Sqrt)
            irs = wpool.tile([P, NW], f32, tag="irs")
            nc.vector.reciprocal(out=irs[:], in_=isq[:])
            iind = wpool.tile([P, NW], f32, tag="iind")
            nc.vector.tensor_scalar_min(out=iind[:], in0=idg[:], scalar1=1.0)
            nd = wpool.tile([P, NW], f32, tag="nd")
            nc.vector.tensor_tensor(out=nd[:], in0=irs[:], in1=iind[:],
                                    op=Alu.mult)

            shard = cpool.tile([P, NW, DOUT], f16)
            rs_flat = rs_int[:].rearrange("r c -> (r c)")
            full = NPC // P  # 97
            nc.sync.dma_start(
                out=shard[:, 0:full, :],
                in_=rs_flat[0:full * P * DOUT].rearrange(
                    "(b p c) -> p b c", p=P, c=DOUT),
            )
            rem = NPC - full * P  # 84
            nc.sync.dma_start(
                out=shard[0:rem, full, :],
                in_=rs_flat[full * P * DOUT:NPC * DOUT].rearrange(
                    "(p c) -> p c", c=DOUT),
            )
            outt = cpool.tile([P, NW, DOUT], f32)
            nc.vector.tensor_tensor(
                out=outt[:], in0=shard[:],
                in1=nd[:].unsqueeze(2).to_broadcast([P, NW, DOUT]),
                op=Alu.mult,
            )
            nc.vector.tensor_tensor(
                out=outt[:], in0=outt[:],
                in1=b_t[:].unsqueeze(1).to_broadcast([P, NW, DOUT]),
                op=Alu.add,
            )
            nc.sync.dma_start(
                out=out_d[0:full * P, :].rearrange("(b p) c -> p b c", p=P),
                in_=outt[:, 0:full, :],
            )
            nc.sync.dma_start(
                out=out_d[full * P:NPC, :],
                in_=outt[0:rem, full, :],
            )

    nc.compile()
    return nc


def _wrap_idx(flat):
    """int16 flat [n] -> [128, n/16] device layout (16-wrap, 8x replicated)."""
    n = len(flat)
    return np.tile(flat.reshape(n // 16, 16).T, (8, 1))


CMAX = 32


def _color_lane(s, row):
    """Per-(core, lane) rank coloring. s: local src (int64), row: dst>>2.

    Returns (perm, mat, counts, ovf_rounds) where mat[r, pos] is the int16
    accum row for main call r (DUMP for holes), counts = per-position lane
    degree (descending), and ovf_rounds is a list of (minpos, maxpos,
    {pos: row}) for spilled edges, rows distinct within each round.

    Colors are constrained to [0, deg(node)) so call r exactly covers the
    deg-sorted prefix. Per-node Kuhn matching, then Kempe-chain repair for
    the rest; true spills go to overflow rounds.
    """
    deg = np.bincount(s, minlength=NPC)
    perm = np.argsort(-deg, kind="stable")
    invp = np.empty(NPC, np.int64)
    invp[perm] = np.arange(NPC)
    pos = invp[s]
    o = np.argsort(pos, kind="stable")
    pos_s = pos[o]
    row_s = row[o]
    counts = deg[perm]
    starts = np.concatenate([[0], np.cumsum(counts)])

    maxdeg = int(counts[0]) if len(counts) else 0
    node_ce = np.full((NPC, CMAX), -1, np.int32)   # pos, color -> row
    row_ce = np.full((R_REAL, CMAX), -1, np.int32)  # row, color -> pos
    row_used = [0] * R_REAL
    spills = []

    for p in range(NPC):
        a, b = int(starts[p]), int(starts[p + 1])
        d = b - a
        if d == 0:
            break
        budget = (1 << d) - 1
        rows_p = [int(row_s[e]) for e in range(a, b)]
        avail = [budget & ~row_used[r] for r in rows_p]
        match_color = [-1] * d            # edge -> color
        color_edge = [-1] * CMAX          # color -> edge

        def try_edge(i, visited):
            m = avail[i] & ~visited
            while m:
                c = (m & -m).bit_length() - 1
                m &= m - 1
                visited |= 1 << c
                if color_edge[c] == -1 or try_edge(color_edge[c], visited):
                    color_edge[c] = i
                    match_color[i] = c
                    return True
            return False

        order = sorted(range(d), key=lambda i: bin(avail[i]).count("1"))
        for i in order:
            free_m = avail[i]
            for e2 in range(d):
                if match_color[e2] >= 0:
                    free_m &= ~(1 << match_color[e2])
            if free_m:
                c = (free_m & -free_m).bit_length() - 1
                color_edge[c] = i
                match_color[i] = c
            elif not try_edge(i, 0):
                spills.append((p, rows_p[i]))
        for i in range(d):
            c = match_color[i]
            if c >= 0:
                node_ce[p, c] = rows_p[i]
                row_ce[rows_p[i], c] = p
                row_used[rows_p[i]] |= 1 << c

    # ---- Kempe-chain repair ----
    degv = counts  # degree by position
    remaining = []
    for (p, r_star) in spills:
        d = int(degv[p])
        a = -1
        for c in range(d):
            if node_ce[p, c] < 0:
                a = c
                break
        if a < 0:
            remaining.append((p, r_star))
            continue
        fixed = False
        for bc in range(maxdeg):
            if bc == a or row_ce[r_star, bc] >= 0:
                continue
            # walk the (a, bc) path from r_star
            path = []
            r = r_star
            valid = False
            guard = 0
            while guard < 100000:
                guard += 1
                u = int(row_ce[r, a])
                if u < 0:
                    valid = True
                    break
                path.append((u, r, a))
                r2 = int(node_ce[u, bc])
                if r2 < 0:
                    valid = bc < int(degv[u])
                    break
                path.append((u, r2, bc))
                r = r2
            if not valid:
                continue
            # flip a <-> b along path
            for (u, rr, col) in path:
                oc = a + bc - col
                node_ce[u, col] = -1
                row_ce[rr, col] = -1
            for (u, rr, col) in path:
                oc = a + bc - col
                node_ce[u, oc] = rr
                row_ce[rr, oc] = u
            node_ce[p, a] = r_star
            row_ce[r_star, a] = p
            fixed = True
            break
        if not fixed:
            remaining.append((p, r_star))

    mat = np.full((max(maxdeg, 1), NSL), DUMP, dtype=np.int16)
    pp, cc = np.nonzero(node_ce[:, :max(maxdeg, 1)] >= 0)
    mat[cc, pp] = node_ce[pp, cc].astype(np.int16)
    return perm, mat, counts, remaining


def _pack_ovf(spills):
    """Pack spills into rounds with distinct rows, position-sorted."""
    rounds = []
    for (p, r) in sorted(spills):
        placed = False
        for rd in rounds:
            if p not in rd[2] and r not in rd[3]:
                rd[2][p] = r
                rd[3].add(r)
                rd[0] = min(rd[0], p)
                rd[1] = max(rd[1], p)
                placed = True
                break
        if not placed:
            rounds.append([p, p, {p: r}, {r}])
    return rounds


def _preprocess(x, edge_index, W, b):
    src = np.asarray(edge_index[0], dtype=np.int64)
    dst = np.asarray(edge_index[1], dtype=np.int64)
    x = np.asarray(x, dtype=np.float32)
    W = np.asarray(W, dtype=np.float32)
    b = np.asarray(b, dtype=np.float32)

    outdeg = np.bincount(src, minlength=N_NODES).astype(np.int32)
    indeg = np.bincount(dst, minlength=N_NODES).astype(np.int32)

    core_of = src // NPC
    lane = dst & 3

    per_core = []  # [k][j] = (perm, mat, counts, ovf_rounds)
    for k in range(N_CORES):
        selc = core_of == k
        s_c = src[selc] - k * NPC
        d_c = dst[selc]
        l_c = lane[selc]
        lanes_k = []
        for j in range(NLANE):
            sel = l_c == j
            lanes_k.append(_color_lane(s_c[sel], (d_c[sel] >> 2)))
        per_core.append(lanes_k)

    # global SPMD-uniform call plan
    MAXB = 32  # max 128-blocks per scatter call (4096-desc hw limit)
    maxdeg_j = [max(per_core[k][j][1].shape[0] for k in range(N_CORES))
                for j in range(NLANE)]
    n_rj = {}
    for j in range(NLANE):
        for r in range(maxdeg_j[j]):
            n_r = 0
            for k in range(N_CORES):
                counts = per_core[k][j][2]
                n_r = max(n_r, int(np.searchsorted(-counts, -(r + 1),
                                                   side="right")))
            n_rj[(j, r)] = n_r

    # tail ranks (n_r <= 128) merge into one replica-block call per lane
    use_tail = os.environ.get("K_TAIL", "1") != "0"
    RT_j = []
    for j in range(NLANE):
        rt = maxdeg_j[j]
        if use_tail:
            for r in range(maxdeg_j[j]):
                if n_rj[(j, r)] <= P:
                    rt = r
                    break
        RT_j.append(rt)
    T_j = [maxdeg_j[j] - RT_j[j] for j in range(NLANE)]

    # move tail-call row collisions into the spill lists (per core)
    for k in range(N_CORES):
        for j in range(NLANE):
            perm, mat, counts, rem = per_core[k][j]
            seen = set()
            for t in range(T_j[j]):
                r = RT_j[j] + t
                if r >= mat.shape[0]:
                    break
                for p in range(P):
                    rr = int(mat[r, p])
                    if rr == DUMP:
                        continue
                    if rr in seen:
                        rem.append((p, rr))
                        mat[r, p] = DUMP
                    else:
                        seen.add(rr)

    # overflow rounds (after tail collisions added)
    ovf = [[_pack_ovf(per_core[k][j][3]) for j in range(NLANE)]
           for k in range(N_CORES)]
    novf_j = [max(len(ovf[k][j]) for k in range(N_CORES))
              for j in range(NLANE)]
    ovf_span = {}
    for j in range(NLANE):
        for t in range(novf_j[j]):
            lo, hi = NSL, 0
            for k in range(N_CORES):
                rds = ovf[k][j]
                if t < len(rds):
                    lo = min(lo, rds[t][0])
                    hi = max(hi, rds[t][1])
            a = lo // P
            nb = (hi // P) + 1 - a
            ovf_span[(j, t)] = (a, nb)

    # plan: main ranks split to <=MAXB blocks, round-robin over lanes,
    # then merged tail calls, then overflow rounds
    plan = []       # (j, a, nb): in slots = m blocks [a, a+nb)
    plan_src = []   # ("rank", j, r, a) | ("tail", j, 0, 0) | ("ovf", j, t, a)
    max_rt = max(RT_j)
    for r in range(max_rt):
        nb_full = {j: (n_rj[(j, r)] + P - 1) // P
                   for j in range(NLANE) if r < RT_j[j]}
        nseg = max((nb + MAXB - 1) // MAXB for nb in nb_full.values())
        for seg in range(nseg):
            for j, nbf in nb_full.items():
                a = seg * MAXB
                if a < nbf:
                    plan.append((j, a, min(MAXB, nbf - a)))
                    plan_src.append(("rank", j, r, a))
    for j in range(NLANE):
        if T_j[j] > 0:
            plan.append((j, NW, T_j[j]))
            plan_src.append(("tail", j, 0, 0))
    for t in range(max(novf_j) if novf_j else 0):
        for j in range(NLANE):
            if t < novf_j[j]:
                a, nb = ovf_span[(j, t)]
                for s in range(0, nb, MAXB):
                    plan.append((j, a + s, min(MAXB, nb - s)))
                    plan_src.append(("ovf", j, t, a + s))
    call_plan = tuple(plan)

    b_rep = np.broadcast_to(b[None, :], (P, DOUT)).copy()

    in_maps = []
    for k in range(N_CORES):
        imap = {"w_in": W, "b_rep": b_rep}
        for j in range(NLANE):
            perm = per_core[k][j][0]
            xs = np.zeros((DIN, NSL), dtype=np.float32)
            xs[:, :NPC] = x[k * NPC + perm].T
            imap[f"xt{j}"] = xs
            dg = np.zeros(NSL, dtype=np.int32)
            dg[:NPC] = outdeg[k * NPC + perm]
            imap[f"odeg{j}"] = dg.reshape(NW, P).T.copy()
        idx_cols = []
        for (kind, j, r, ac), (jj, a, nb) in zip(plan_src, call_plan):
            mat = per_core[k][j][1]
            if kind == "rank":
                if r < mat.shape[0]:
                    fl = mat[r, a * P:(a + nb) * P].copy()
                else:
                    fl = np.full(nb * P, DUMP, np.int16)
            elif kind == "tail":
                fl = np.full(nb * P, DUMP, np.int16)
                for t in range(nb):
                    rr = RT_j[j] + t
                    if rr < mat.shape[0]:
                        fl[t * P:(t + 1) * P] = mat[rr, 0:P]
            else:
                fl = np.full(nb * P, DUMP, np.int16)
                rds = ovf[k][j]
                if r < len(rds):
                    for p0, rr in rds[r][2].items():
                        if a * P <= p0 < (a + nb) * P:
                            fl[p0 - a * P] = rr
            idx_cols.append(_wrap_idx(fl.astype(np.int16)))
        imap["idx_in"] = np.ascontiguousarray(
            np.concatenate(idx_cols, axis=1))
        idg = np.zeros(NSL, dtype=np.int32)
        idg[:NPC] = indeg[k * NPC:(k + 1) * NPC]
        imap["ideg_in"] = idg.reshape(NW, P).T.copy()
        in_maps.append(imap)

    return in_maps, call_plan


_prog_cache = {}
_last_results = None


def kernel(x, edge_index, W, b):
    from concourse import bass_utils

    in_maps, call_plan = _preprocess(x, edge_index, W, b)
    if call_plan not in _prog_cache:
        _prog_cache[call_plan] = _build_program(call_plan)
    nc = _prog_cache[call_plan]

    res = bass_utils.run_bass_kernel_spmd(
        nc, in_maps, core_ids=list(range(N_CORES))
    )
    global _last_results
    _last_results = res
    outs = []
    for k in range(N_CORES):
        outs.append(res.results[k]["out_d"])
    return np.concatenate(outs, axis=0).astype(np.float32)
